# revision 15
# baseline (speedup 1.0000x reference)
"""CapsNet routing layer (grouped 1x1 conv + 3 dynamic-routing iterations)
as a Bass/Tile kernel on 8 Trainium2 NeuronCores.

Contract: kernel(**inputs) takes FULL unsharded inputs
    x        (256, 256, 6, 6)  f32
    W_weight (32, 160, 8)      f32
    W_bias   (5120,)           f32
and returns FULL output v (256, 10, 16) f32.

Strategy: data-parallel over batch (32 per core).  pred is never
materialized: all routing contractions use the rank-9 factorization
pred = W' @ xg' (bias folded in as a 9th input channel of ones).

Per-core on-device pipeline per routing iteration:
  step-u : u = v . W'            (PE, W' stationary, block-structured v)
  upd    : logits += u . xg'     (PE, block-diag xg' rhs, col-tiled out)
  exp    : L-add (DVE) -> PE-transpose -> exp (ACT) -> eT tiles
  y      : Y = xg'^T . e^T       (PE, block-diag lhsT, K-chunk accum)
  s      : s = W2' . Y           (PE) ; squash + softmax-denominator (DVE)
"""

import contextlib

import numpy as np
import ml_dtypes

import concourse.bass as bass
import concourse.tile as tile
import concourse.mybir as mybir
from concourse.bass_utils import run_bass_kernel_spmd

BF16 = ml_dtypes.bfloat16
F32 = np.float32
EPS = float(np.finfo(np.float64).eps)

N_CORES = 8
NB = 32          # batch per core
G, J, D, IN = 32, 10, 16, 8
IP = 9           # in-dim + ones channel
S = 36           # spatial
NBLK, GB = 4, 8  # 4 g-blocks of 8
KGI = GB * IP    # 72
NS = GB * S      # 288
DP = 32          # d padded to 32
ROUTE_NUM = 3

_AF = mybir.ActivationFunctionType

# XBD2 row-chunking of the 288 (gh, s) rows into 128/128/32,
# as (chunk, local_row, gh, s_start, s_count) segments
_XBD2_SEGS = [
    (0, 0, 0, 0, 36), (0, 36, 1, 0, 36), (0, 72, 2, 0, 36), (0, 108, 3, 0, 20),
    (1, 0, 3, 20, 16), (1, 16, 4, 0, 36), (1, 52, 5, 0, 36), (1, 88, 6, 0, 36),
    (1, 124, 7, 0, 4), (2, 0, 7, 4, 32),
]
_CHUNKS = (128, 128, 32)   # XBD2 / eT row chunks of 288
_VCH = (128, 128, 64)      # vbd / W1T row chunks of 320 = (J, DP)


def _split_waits(nc, limit=1):
    """walrus on this toolchain rejects >1 sync-wait per instruction
    (TPB_CTRL setupSyncWait); split extras onto preceding NoOps."""
    for f in nc.m.functions:
        for bb in f.blocks:
            insts = bb.instructions
            i = 0
            while i < len(insts):
                inst = insts[i]
                si = inst.sync_info
                if si is not None and si.on_wait and len(si.on_wait) > limit:
                    waits = list(si.on_wait)
                    keep = waits[-limit:]
                    extra = waits[:-limit]
                    si.on_wait = keep
                    pos = i
                    while extra:
                        chunk, extra = extra[:limit], extra[limit:]
                        nop = mybir.InstNoOp(
                            name=nc.get_next_instruction_name(),
                            sync_info=mybir.SyncInfo(on_wait=chunk, on_update=[]),
                            bass_nofuse=True,
                            engine=inst.engine,
                        )
                        nc.register_instruction(nop, overwrite=True)
                        insts.insert(pos, nop)
                        pos += 1
                        i += 1
                i += 1


def _build_program():
    nc = bass.Bass()
    bf = mybir.dt.bfloat16
    f32 = mybir.dt.float32

    # ---- dram tensors (per core) ----
    d_xbdg = nc.dram_tensor("xbdg_src", [KGI, NB * NBLK * NS], bf, kind="ExternalInput")
    d_xbd2 = [
        nc.dram_tensor(f"xbd2_src{c}", [_CHUNKS[c], NB * NBLK * KGI], bf, kind="ExternalInput")
        for c in range(3)
    ]
    d_zeros = nc.dram_tensor("zeros_src", [128, 2 * NS], bf, kind="ExternalInput")
    d_xgd = nc.dram_tensor("xgd_src", [NBLK * KGI, NB * S], bf, kind="ExternalInput")
    d_w1t = nc.dram_tensor("w1t_src", [J * DP, G * IP], bf, kind="ExternalInput")
    d_w2t = nc.dram_tensor("w2t_src", [NBLK * KGI, J * DP], bf, kind="ExternalInput")
    d_id32 = nc.dram_tensor("id32_src", [32, 32], f32, kind="ExternalInput")
    d_id128 = nc.dram_tensor("id128_src", [128, 128], bf, kind="ExternalInput")
    d_out = nc.dram_tensor("v_out", [NB, J, D], f32, kind="ExternalOutput")

    with tile.TileContext(nc) as tc, contextlib.ExitStack() as ctx:
        cpool = ctx.enter_context(tc.tile_pool(name="consts", bufs=1))
        lpool = ctx.enter_context(tc.tile_pool(name="logits", bufs=1))
        wpool = ctx.enter_context(tc.tile_pool(name="work", bufs=1))
        psp = ctx.enter_context(tc.tile_pool(name="psp", bufs=2, space="PSUM"))

        # ================= constants into SBUF =================
        # emission order = DMA queue order: it0 deps first, y deps last
        xgd = [cpool.tile([_CHUNKS[c], NB * S], bf, tag=f"xgd{c}", name=f"xgd{c}") for c in range(3)]
        w2tf = [cpool.tile([_CHUNKS[c], J * DP], bf, tag=f"w2tf{c}", name=f"w2tf{c}") for c in range(3)]
        off = 0
        for c in range(3):
            nc.sync.dma_start(xgd[c][:], d_xgd[off : off + _CHUNKS[c]])
            nc.sync.dma_start(w2tf[c][:], d_w2t[off : off + _CHUNKS[c]])
            off += _CHUNKS[c]
        w1t = [cpool.tile([_VCH[c], G * IP], bf, tag=f"w1t{c}", name=f"w1t{c}") for c in range(3)]
        off = 0
        for c in range(3):
            nc.sync.dma_start(w1t[c][:], d_w1t[off : off + _VCH[c]])
            off += _VCH[c]
        id32 = cpool.tile([32, 32], f32, tag="id32", name="id32")
        nc.sync.dma_start(id32[:], d_id32[:])
        id128 = cpool.tile([128, 128], bf, tag="id128", name="id128")
        nc.sync.dma_start(id128[:], d_id128[:])

        # zero-padded block-diag tensors, host-built, contiguous split DMAs
        xbdg = cpool.tile([KGI, NB * NBLK * NS], bf, tag="xbdg", name="xbdg")
        nsplit = 16
        w = NB * NBLK * NS // nsplit
        for i in range(nsplit):
            nc.sync.dma_start(
                xbdg[:, i * w : (i + 1) * w], d_xbdg[:, i * w : (i + 1) * w]
            )
        xbd2 = [
            cpool.tile([_CHUNKS[c], NB * NBLK * KGI], bf, tag=f"xbd2{c}", name=f"xbd2{c}")
            for c in range(3)
        ]
        for c in range(3):
            w2 = NB * NBLK * KGI // 8
            for i in range(8):
                nc.sync.dma_start(
                    xbd2[c][:, i * w2 : (i + 1) * w2],
                    d_xbd2[c][:, i * w2 : (i + 1) * w2],
                )

        # ================= working tensors =================
        ltiles = [lpool.tile([128, 2 * NS], bf, tag=f"L{t}", name=f"L{t}") for t in range(16)]
        for t in range(16):
            nc.sync.dma_start(ltiles[t][:], d_zeros[:])
        u_sb = [wpool.tile([KGI, NB * DP], bf, tag=f"u{B}", name=f"u{B}") for B in range(NBLK)]
        for B in range(NBLK):
            nc.vector.memset(u_sb[B][:], 0.0)
        vbd = [wpool.tile([_VCH[c], J * NB], bf, tag=f"vbd{c}", name=f"vbd{c}") for c in range(3)]
        for c in range(3):
            nc.vector.memset(vbd[c][:], 0.0)
        et = [wpool.tile([_CHUNKS[c], 40 * NB], bf, tag=f"et{c}", name=f"et{c}") for c in range(3)]
        ytc = [wpool.tile([_CHUNKS[c], NB * J], bf, tag=f"ytc{c}", name=f"ytc{c}") for c in range(3)]
        ysb2 = [wpool.tile([128, NBLK * KGI], bf, tag=f"yg{q}", name=f"yg{q}") for q in range(8)]
        spad = wpool.tile([128, 4 * NB], f32, tag="spad", name="spad")
        vpre = wpool.tile([NB, J * DP], f32, tag="vpre", name="vpre")
        vpad = wpool.tile([NB, J * DP], f32, tag="vpad", name="vpad")
        nc.vector.memset(vpad[:], 0.0)
        sq = wpool.tile([NB, J * 16], f32, tag="sq", name="sq")
        xsum = [wpool.tile([_CHUNKS[c], NB], f32, tag=f"xsum{c}", name=f"xsum{c}") for c in range(3)]
        sc = {
            n: wpool.tile([NB, J], f32, tag=f"sc_{n}", name=f"sc_{n}")
            for n in ("n2r", "den", "r1", "n2t", "nt", "t0", "t1", "r2", "f")
        }

        # ================= it0: uniform-c shortcut =================
        for c in range(3):
            xg3 = xgd[c][:].rearrange("p (b s) -> p b s", b=NB)
            nc.vector.reduce_sum(xsum[c][:], xg3, axis=mybir.AxisListType.X)
            yv = ytc[c][:].rearrange("p (b j) -> p b j", b=NB)
            for j in range(J):
                nc.scalar.copy(yv[:, :, j], xsum[c][:])

        def s_step_and_squash(last_iter):
            # s matmuls: out [32=(d16+den+pad), 32 b] per j, accum over B
            # (PSUM out base partition limited to 0/32/64 -> stack 3 j per col)
            stile = psp.tile([128, 4 * NB], mybir.dt.float32, tag="pp", name="sps")
            for j in range(J):
                tj, rj = divmod(j, 3)
                outap = stile[32 * rj : 32 * rj + 32, 32 * tj : 32 * tj + 32]
                for c in range(3):
                    yap = ytc[c][:].rearrange("p (b j) -> p j b", j=J)
                    nc.tensor.matmul(
                        outap,
                        lhsT=w2tf[c][:, DP * j : DP * j + DP],
                        rhs=yap[:, j],
                        start=(c == 0),
                        stop=(c == 2),
                    )
            nc.vector.tensor_copy(spad[:], stile[:])
            vprev = vpre[:].rearrange("p (j q) -> p j q", q=DP)
            for j in range(J):
                tj, rj = divmod(j, 3)
                nc.vector.transpose(
                    vprev[:, j],
                    spad[32 * rj : 32 * rj + 32, 32 * tj : 32 * tj + 32],
                )
            # n2r = sum_d s_raw^2 ; den at dp=16
            sqv = sq[:].rearrange("p (j q) -> p j q", q=16)
            nc.vector.tensor_mul(sqv, vprev[:, :, 0:16], vprev[:, :, 0:16])
            nc.vector.reduce_sum(sc["n2r"][:], sqv, axis=mybir.AxisListType.X)
            nc.vector.tensor_copy(sc["den"][:], vprev[:, :, 16])
            nc.vector.reciprocal(sc["r1"][:], sc["den"][:])
            nc.vector.tensor_mul(sc["t0"][:], sc["n2r"][:], sc["r1"][:])
            nc.vector.tensor_mul(sc["n2t"][:], sc["t0"][:], sc["r1"][:])
            nc.scalar.sqrt(sc["nt"][:], sc["n2r"][:])
            nc.vector.tensor_mul(sc["nt"][:], sc["nt"][:], sc["r1"][:])
            nc.vector.tensor_scalar_add(sc["t0"][:], sc["n2t"][:], 1.0)
            nc.vector.tensor_scalar_add(sc["t1"][:], sc["nt"][:], EPS)
            nc.vector.tensor_mul(sc["t0"][:], sc["t0"][:], sc["t1"][:])
            nc.vector.reciprocal(sc["r2"][:], sc["t0"][:])
            nc.vector.tensor_mul(sc["r2"][:], sc["r2"][:], sc["n2t"][:])
            nc.vector.tensor_mul(sc["f"][:], sc["r2"][:], sc["r1"][:])
            for j in range(J):
                nc.vector.tensor_scalar_mul(
                    vpad[:, DP * j : DP * j + 16],
                    vprev[:, j, 0:16],
                    sc["f"][:, j : j + 1],
                )
            if last_iter:
                vpadv = vpad[:].rearrange("p (j q) -> p j q", q=DP)
                nc.sync.dma_start(d_out[:], vpadv[:, :, 0:16])

        s_step_and_squash(last_iter=False)

        # ================= routing iterations =================
        for it in range(1, ROUTE_NUM):
            # ---- vT + vbd ----
            vtp = psp.tile([128, 3 * NB], mybir.dt.float32, tag="pp", name="vtp")
            off = 0
            for c in range(3):
                nc.tensor.transpose(
                    vtp[: _VCH[c], 32 * c : 32 * c + 32],
                    vpad[:, off : off + _VCH[c]],
                    id32[:],
                )
                off += _VCH[c]
            for j in range(J):
                tj, rj = divmod(j, 4)
                nc.vector.tensor_copy(
                    vbd[tj][32 * rj : 32 * rj + 32, 32 * j : 32 * j + 32],
                    vtp[32 * rj : 32 * rj + 32, 32 * tj : 32 * tj + 32],
                )
            # ---- step-u ----
            for B in range(NBLK):
                up = psp.tile([KGI, J * NB], mybir.dt.float32, tag="uy", name="up")
                for c in range(3):
                    nc.tensor.matmul(
                        up[:],
                        lhsT=w1t[c][:, KGI * B : KGI * B + KGI],
                        rhs=vbd[c][:],
                        start=(c == 0),
                        stop=(c == 2),
                    )
                uv = u_sb[B][:].rearrange("p (b q) -> p b q", b=NB)
                upv = up[:].rearrange("p (j b) -> p b j", j=J)
                nc.vector.tensor_copy(uv[:, :, 0:J], upv[:])
            # ---- upd + L add + transpose + exp ----
            # PSUM out base limited to 0/32/64: B=0..2 stack in lpa, B=3 in lpb
            for t in range(16):  # 16 L tiles, 2 b each
                for h in range(2):
                    b = 2 * t + h
                    lp = psp.tile([128, 1024], mybir.dt.float32, tag="lp", name="lp")
                    for B in range(NBLK):
                        outap = (
                            lp[32 * B : 32 * B + 32, 0:NS]
                            if B < 3
                            else lp[0:32, 512 : 512 + NS]
                        )
                        nc.tensor.matmul(
                            outap,
                            lhsT=u_sb[B][:, DP * b : DP * b + DP],
                            rhs=xbdg[
                                :,
                                (NBLK * NS) * b + NS * B : (NBLK * NS) * b + NS * B + NS,
                            ],
                            start=True,
                            stop=True,
                        )
                    nc.vector.tensor_add(
                        ltiles[t][0:96, NS * h : NS * h + NS],
                        lp[0:96, 0:NS],
                        ltiles[t][0:96, NS * h : NS * h + NS],
                    )
                    nc.vector.tensor_add(
                        ltiles[t][96:128, NS * h : NS * h + NS],
                        lp[0:32, 512 : 512 + NS],
                        ltiles[t][96:128, NS * h : NS * h + NS],
                    )
                for h in range(2):
                    b = 2 * t + h
                    off = 0
                    for c in range(3):
                        etp = psp.tile([128, 128], mybir.dt.bfloat16, tag="pp", name="etp")
                        nc.tensor.transpose(
                            etp[: _CHUNKS[c], :],
                            ltiles[t][:, NS * h + off : NS * h + off + _CHUNKS[c]],
                            id128[:],
                        )
                        off += _CHUNKS[c]
                        epv = etp[: _CHUNKS[c], :].rearrange(
                            "p (k q) -> p k q", k=4
                        )
                        etv = et[c][: _CHUNKS[c], 40 * b : 40 * b + 40].rearrange(
                            "p (k j) -> p k j", k=4
                        )
                        nc.scalar.activation(etv, epv[:, :, 0:J], _AF.Exp)
            # ---- y ----
            for b in range(NB):
                yb = psp.tile([J, NBLK * KGI], mybir.dt.float32, tag="uy", name="yb")
                for B in range(NBLK):
                    for c in range(3):
                        nc.tensor.matmul(
                            yb[:, KGI * B : KGI * B + KGI],
                            lhsT=et[c][:, 40 * b + 10 * B : 40 * b + 10 * B + J],
                            rhs=xbd2[c][
                                :,
                                KGI * (NBLK * b + B) : KGI * (NBLK * b + B) + KGI,
                            ],
                            start=(c == 0),
                            stop=(c == 2),
                        )
                dst = ysb2[b // 4][32 * (b % 4) : 32 * (b % 4) + 10, :]
                if b % 2:
                    nc.scalar.copy(dst, yb[:])
                else:
                    nc.vector.tensor_copy(dst, yb[:])
            # transpose Y [40,(B,gh,ip)] -> ytc [(B,gh,ip)-chunks, (b,j)]
            for q in range(8):
                off = 0
                for c in range(3):
                    ytp = psp.tile([128, 128], mybir.dt.bfloat16, tag="pp", name="ytp")
                    nc.tensor.transpose(
                        ytp[: _CHUNKS[c], :],
                        ysb2[q][:, off : off + _CHUNKS[c]],
                        id128[:],
                    )
                    off += _CHUNKS[c]
                    ytv = ytp[: _CHUNKS[c], :].rearrange("p (q r) -> p q r", q=4)
                    dyv = ytc[c][:, 40 * q : 40 * q + 40].rearrange(
                        "p (q r) -> p q r", q=4
                    )
                    nc.vector.tensor_copy(dyv, ytv[:, :, 0:J])
            s_step_and_squash(last_iter=(it == ROUTE_NUM - 1))

    _split_waits(nc)
    return nc


def _host_prep(x, W_weight, W_bias):
    """Build per-core input maps (layout/cast only)."""
    xg = x.reshape(256, G, IN, S).astype(F32)
    xgp = np.concatenate([xg, np.ones((256, G, 1, S), F32)], axis=2)  # b g ip s

    Wp = np.concatenate([W_weight, W_bias.reshape(G, J * D, 1)], axis=2).astype(F32)
    WpR = Wp.reshape(G, J, D, IP)

    # w1t [(j,dp), (g,ip)]
    w1t = np.zeros((J * DP, G * IP), F32)
    for j in range(J):
        w1t[DP * j : DP * j + D, :] = (
            WpR[:, j].transpose(1, 0, 2).reshape(D, G * IP)
        )
    # w2tf [(B,gh,ip) flat 288, (j,dp)] with den marker at dp=16
    w2t = np.zeros((NBLK * KGI, J * DP), F32)
    for g in range(G):
        for j in range(J):
            w2t[IP * g : IP * g + IP, DP * j : DP * j + D] = WpR[g, j].T
            w2t[IP * g + 8, DP * j + 16] = 1.0
    id32 = np.eye(32, dtype=F32)
    id128 = np.eye(128, dtype=BF16)

    w1tb = w1t.astype(BF16)
    w2tb = w2t.astype(BF16)
    zeros = np.zeros((128, 2 * NS), BF16)
    in_maps = []
    for c in range(N_CORES):
        xb = xgp[c * NB : (c + 1) * NB]  # (NB, G, IP, S)
        x5 = xb.reshape(NB, NBLK, GB, IP, S).astype(BF16)
        # xbdg_src: full zero-padded [KGI, (b, B, (gh', s))]
        xbdg_src = np.zeros((KGI, NB, NBLK, GB, S), BF16)
        for gh in range(GB):
            xbdg_src[IP * gh : IP * gh + IP, :, :, gh, :] = x5[:, :, gh].transpose(
                2, 0, 1, 3
            )
        xbdg_src = xbdg_src.reshape(KGI, NB * NBLK * NS)
        # xbd2_src chunks: full zero-padded [(gh,s)-chunk, (b, B, (gh', ip))]
        xbd2_full = np.zeros((GB * S, NB, NBLK, GB, IP), BF16)
        for gh in range(GB):
            xbd2_full[S * gh : S * gh + S, :, :, gh, :] = x5[:, :, gh].transpose(
                3, 0, 1, 2
            )
        xbd2_full = xbd2_full.reshape(GB * S, NB * NBLK * KGI)
        xbd2_srcs = [
            np.ascontiguousarray(xbd2_full[0:128]),
            np.ascontiguousarray(xbd2_full[128:256]),
            np.ascontiguousarray(xbd2_full[256:288]),
        ]
        xgd_src = (
            np.ascontiguousarray(x5.transpose(1, 2, 3, 0, 4))
            .reshape(NBLK * KGI, NB * S)
            .astype(BF16)
        )
        in_maps.append(
            {
                "xbdg_src": xbdg_src,
                "xbd2_src0": xbd2_srcs[0],
                "xbd2_src1": xbd2_srcs[1],
                "xbd2_src2": xbd2_srcs[2],
                "xgd_src": xgd_src,
                "w1t_src": w1tb,
                "w2t_src": w2tb,
                "id32_src": id32,
                "id128_src": id128,
                "zeros_src": zeros,
            }
        )
    return in_maps


_NC_CACHE = {}


def kernel(x, W_weight, W_bias):
    x = np.asarray(x, F32)
    W_weight = np.asarray(W_weight, F32)
    W_bias = np.asarray(W_bias, F32)
    in_maps = _host_prep(x, W_weight, W_bias)
    if "nc" not in _NC_CACHE:
        _NC_CACHE["nc"] = _build_program()
    nc = _NC_CACHE["nc"]
    res = run_bass_kernel_spmd(nc, in_maps, core_ids=list(range(N_CORES)))
    out = np.concatenate([res.results[i]["v_out"] for i in range(N_CORES)], axis=0)
    return out.astype(F32)
